# revision 13
# baseline (speedup 1.0000x reference)
"""Cached multi-head attention, sharded over heads across 8 TRN2 NeuronCores.

Per-core work (2 of 16 heads, all 8 batches):
  qkv^T via Wqkv-stationary matmuls producing [outdim, token] layout;
  per (batch, head): scores^T[k, q] with k^T-stationary matmuls (fp16),
  exp on ACT (no max subtraction: scores are O(1) by construction),
  causal mask on the last 128-key block, softmax denominator via a
  ones-stationary matmul pass, attn@V with v-stationary accumulating
  out^T[d, q], normalization through a reciprocal partition-broadcast,
  then the output projection emits a partial final^T [2048, 1024];
  the host sums the 8 partials and adds the bias.

Layout notes:
  - k cache is host-transposed to [h, b, HD, CACHE] fp16 (line-rate DMA).
  - v cache is pair-packed on host ([16, 128, 256] fp16 per (h,b): two
    128-key chunks side by side in the free dim) so DMA runs are 512B.
  - QKV runs in two token halves so batch 0-3 attention starts early.
  - The output projection runs on 256-token blocks as batches finish.
"""

import numpy as np

import concourse.bacc as bacc
import concourse.mybir as mybir
import concourse.tile as tile
from concourse.bass_utils import run_bass_kernel_spmd

B, Q, D = 8, 128, 2048
H, HD = 16, 128
CACHE = 3968
K = CACHE + Q          # 4096
NCORES = 8
HPC = H // NCORES      # heads per core
NKC = K // 128         # 32 key chunks
NC2 = NKC // 2         # 16 packed chunk-pairs
TOK = B * Q            # 1024 tokens
QKV_COLS = 3 * HPC * HD  # 768 per core
SCALE = 1.0 / float(np.sqrt(HD))

F16 = mybir.dt.float16
F32 = mybir.dt.float32

_STATE = {}


def build_nc():
    nc = bacc.Bacc("TRN2", target_bir_lowering=False, debug=False)

    xt_d = nc.dram_tensor("xt", [D, TOK], F16, kind="ExternalInput")
    wq_d = nc.dram_tensor("wqkv", [D, QKV_COLS], F16, kind="ExternalInput")
    kt_d = nc.dram_tensor("kt", [HPC, B, HD, CACHE], F16, kind="ExternalInput")
    vp_d = nc.dram_tensor("vp", [HPC, B, NC2, 128, 256], F16, kind="ExternalInput")
    wp_d = nc.dram_tensor("wp", [HPC * HD, D], F16, kind="ExternalInput")
    out_d = nc.dram_tensor("out", [D, TOK], F16, kind="ExternalOutput")

    with tile.TileContext(nc) as tc:
        with (
            tc.tile_pool(name="const", bufs=1) as cpool,
            tc.tile_pool(name="xw", bufs=1) as xwpool,
            tc.tile_pool(name="qkv", bufs=1) as qkvpool,
            tc.tile_pool(name="vnew", bufs=1) as vnewpool,
            tc.tile_pool(name="attn", bufs=1) as attnpool,
            tc.tile_pool(name="kt", bufs=6) as ktpool,
            tc.tile_pool(name="v", bufs=6) as vpool,
            tc.tile_pool(name="p", bufs=2) as ppool,
            tc.tile_pool(name="fold", bufs=2) as foldpool,
            tc.tile_pool(name="small", bufs=2) as smallpool,
            tc.tile_pool(name="ostage", bufs=3) as opool,
            tc.tile_pool(name="ps_s", bufs=2, space="PSUM") as psum_s,
            tc.tile_pool(name="ps_proj", bufs=1, space="PSUM") as psum_p,
            tc.tile_pool(name="ps_o", bufs=2, space="PSUM") as psum_o,
            tc.tile_pool(name="ps_d", bufs=1, space="PSUM") as psum_d,
        ):
            # constants
            ones_col = cpool.tile([128, 1], F16)       # denominator stationary
            nc.vector.memset(ones_col[:], 1.0)
            ones_full = cpool.tile([128, 128], F16)
            nc.vector.memset(ones_full[:], 1.0)
            # causal mask for the last key block: keep (p=key j', free=query i)
            # where i >= j'  -> iota = i - j' >= 0
            zeros_full = cpool.tile([128, 128], F32)
            nc.vector.memset(zeros_full[:], 0.0)
            maskneg = cpool.tile([128, 128], F32)
            nc.gpsimd.affine_select(
                maskneg[:], zeros_full[:], pattern=[[1, 128]],
                compare_op=mybir.AluOpType.is_ge, fill=-1e30,
                base=0, channel_multiplier=-1,
            )
            ident = cpool.tile([128, 128], F16)
            nc.gpsimd.affine_select(
                ident[:], ones_full[:], pattern=[[1, 128]],
                compare_op=mybir.AluOpType.is_equal, fill=0.0,
                base=0, channel_multiplier=-1,
            )

            wq_sb = xwpool.tile([128, D // 128, QKV_COLS], F16)
            xt_r = xt_d.ap().rearrange("(t p) n -> p t n", p=128)
            wq_r = wq_d.ap().rearrange("(t p) c -> p t c", p=128)
            xt_half = [None, None]

            def load_xt(t):
                xt_half[t] = xwpool.tile([128, D // 128, 512], F16,
                                         tag="xt", name=f"xt{t}")
                for dh in range(2):
                    dsl = slice(dh * 8, (dh + 1) * 8)
                    nc.sync.dma_start(
                        xt_half[t][:, dsl, :],
                        xt_r[:, dsl, t * 512:(t + 1) * 512],
                    )

            load_xt(0)
            for dh in range(2):
                dsl = slice(dh * 8, (dh + 1) * 8)
                nc.sync.dma_start(wq_sb[:, dsl, :], wq_r[:, dsl, :])

            qkvT = [
                qkvpool.tile([128, QKV_COLS // 128, 512], F16,
                             tag=f"qkvT{t}", name=f"qkvT{t}")
                for t in range(2)
            ]
            vnew_sb = vnewpool.tile([128, HPC, B, HD], F16)

            def qkv_group(t, oc):
                ps = psum_s.tile([128, 512], F32, tag="ps_s", name="ps_qkv")
                for dt_ in range(D // 128):
                    nc.tensor.matmul(
                        ps[:],
                        wq_sb[:, dt_, oc * 128:(oc + 1) * 128],
                        xt_half[t][:, dt_, :],
                        start=(dt_ == 0), stop=(dt_ == D // 128 - 1),
                    )
                nc.scalar.copy(qkvT[t][:, oc, :], ps[:])

            def vnew_transposes(t, hh):
                # v_new natural layout via PE transpose for this half's batches
                for bb in range(4):
                    b = 4 * t + bb
                    ps_t = psum_o.tile([128, 128], F16, tag="ps_o")
                    nc.tensor.transpose(
                        ps_t[:], qkvT[t][:, 2 * HPC + hh, bb * 128:(bb + 1) * 128],
                        ident[:],
                    )
                    nc.scalar.copy(vnew_sb[:, hh, b, :], ps_t[:])

            def qkv_head(t, hh):
                for oc in (hh, HPC + hh, 2 * HPC + hh):
                    qkv_group(t, oc)
                vnew_transposes(t, hh)

            qkv_head(0, 0)

            attn_sb = attnpool.tile([128, HPC, TOK], F16)
            wp_sb = xwpool.tile([128, HPC, D], F16)
            out_r = out_d.ap().rearrange("(cc p) n -> cc p n", p=128)

            def proj_block(tb):
                # partial final^T for 256 tokens (batches 2tb, 2tb+1)
                for cc in range(D // 128):
                    ps = psum_p.tile([128, 256], F32, tag="ps_p")
                    for ht in range(HPC):
                        nc.tensor.matmul(
                            ps[:],
                            wp_sb[:, ht, cc * 128:(cc + 1) * 128],
                            attn_sb[:, ht, tb * 256:(tb + 1) * 256],
                            start=(ht == 0), stop=(ht == HPC - 1),
                        )
                    o_sb = opool.tile([128, 256], F16)
                    if cc % 4 != 1:
                        nc.vector.tensor_copy(o_sb[:], ps[:])
                    else:
                        nc.scalar.copy(o_sb[:], ps[:])
                    nc.sync.dma_start(out_r[cc, :, tb * 256:(tb + 1) * 256], o_sb[:])

            for b in range(B):
                t, bb = divmod(b, 4)
                for hh in range(HPC):
                    if b == 0 and hh == 1:
                        qkv_head(0, 1)
                    kt_sb = ktpool.tile([128, K], F16)
                    nc.sync.dma_start(kt_sb[:, 0:CACHE], kt_d.ap()[hh, b])
                    nc.vector.tensor_copy(
                        kt_sb[:, CACHE:K], qkvT[t][:, HPC + hh, bb * 128:(bb + 1) * 128]
                    )
                    v_sb = vpool.tile([128, NC2, 256], F16)
                    nc.sync.dma_start(
                        v_sb[:], vp_d.ap()[hh, b].rearrange("c k d -> k c d")
                    )
                    nc.vector.tensor_copy(
                        v_sb[:, NC2 - 1, 128:256], vnew_sb[:, hh, b, :]
                    )

                    qT = qkvT[t][:, hh, bb * 128:(bb + 1) * 128]
                    pT = ppool.tile([128, K], F16)
                    ps_o = psum_o.tile([128, 128], F32, tag="ps_o")
                    for g in range(4):
                        ps = psum_s.tile([128, 1024], F32, tag="ps_s")
                        for j in range(8):
                            kc = g * 8 + j
                            nc.tensor.matmul(
                                ps[:, j * 128:(j + 1) * 128],
                                kt_sb[:, kc * 128:(kc + 1) * 128],
                                qT,
                                start=True, stop=True,
                            )
                        if g == 3:
                            # causal mask on the new-key block (pre-exp)
                            nc.vector.tensor_add(
                                ps[:, 896:1024], ps[:, 896:1024], maskneg[:]
                            )
                        pslab = pT[:, g * 1024:(g + 1) * 1024]
                        nc.scalar.activation(
                            pslab, ps[:],
                            mybir.ActivationFunctionType.Exp, scale=SCALE,
                        )
                        for j in range(8):
                            c = g * 8 + j
                            c2, jj = divmod(c, 2)
                            nc.tensor.matmul(
                                ps_o[:], v_sb[:, c2, jj * 128:(jj + 1) * 128],
                                pT[:, c * 128:(c + 1) * 128],
                                start=(c == 0), stop=(c == NKC - 1),
                            )
                    # denominator: log2 fold on DVE, then ones-matmul over partitions
                    fold = foldpool.tile([128, 2048], F16)
                    nc.vector.tensor_add(fold[:, 0:2048], pT[:, 0:2048], pT[:, 2048:K])
                    for w in (1024, 512, 256, 128):
                        nc.vector.tensor_add(
                            fold[:, 0:w], fold[:, 0:w], fold[:, w:2 * w]
                        )
                    ps_d = psum_d.tile([1, 128], F32, tag="ps_d")
                    nc.tensor.matmul(
                        ps_d[:], ones_col[:], fold[:, 0:128], start=True, stop=True
                    )
                    inv_d = smallpool.tile([1, 128], F32, tag="inv")
                    nc.vector.reciprocal(inv_d[:], ps_d[:])
                    bcast = smallpool.tile([128, 128], F32, tag="bc")
                    nc.gpsimd.partition_broadcast(bcast[:], inv_d[:])
                    nc.vector.tensor_mul(
                        attn_sb[:, hh, b * 128:(b + 1) * 128], ps_o[:], bcast[:]
                    )

                if b == 0:
                    load_xt(1)
                    nc.sync.dma_start(
                        wp_sb[:], wp_d.ap().rearrange("(t p) c -> p t c", p=128)
                    )
                elif b == 1:
                    qkv_head(1, 0)
                elif b == 2:
                    qkv_head(1, 1)
                    proj_block(0)
                elif b in (3, 5):
                    proj_block(b // 2)
            proj_block(3)

    nc.compile()
    return nc


def prepare_in_maps(x, k_cache, v_cache, Wqkv, Wproj):
    xt = np.ascontiguousarray(x.reshape(TOK, D).T, dtype=np.float16)
    in_maps = []
    for c in range(NCORES):
        h0 = c * HPC
        cols = []
        for i3 in range(3):
            for hh in range(HPC):
                h = h0 + hh
                cols.append(Wqkv[:, i3 * D + h * HD:(i3 * D + (h + 1) * HD)])
        wq = np.ascontiguousarray(np.concatenate(cols, axis=1), dtype=np.float16)
        ks = k_cache[:, h0:h0 + HPC]                  # [B, HPC, CACHE, HD]
        kt = np.ascontiguousarray(
            np.transpose(ks, (1, 0, 3, 2)), dtype=np.float16
        )                                             # [HPC, B, HD, CACHE]
        vs = v_cache[:, h0:h0 + HPC].astype(np.float16)  # [B, HPC, CACHE, HD]
        vp = np.zeros((HPC, B, NC2, 128, 256), np.float16)
        full = vs[:, :, :NC2 * 256 - 256, :].reshape(B, HPC, NC2 - 1, 2, 128, HD)
        vp[:, :, :NC2 - 1] = np.transpose(full, (1, 0, 2, 4, 3, 5)).reshape(
            HPC, B, NC2 - 1, 128, 256
        )
        vp[:, :, NC2 - 1, :, 0:128] = np.transpose(
            vs[:, :, NC2 * 256 - 256:, :], (1, 0, 2, 3)
        )
        wp = np.ascontiguousarray(
            Wproj[h0 * HD:(h0 + HPC) * HD, :], dtype=np.float16
        )
        in_maps.append({"xt": xt, "wqkv": wq, "kt": kt, "vp": vp, "wp": wp})
    return in_maps


def postprocess(results, bproj):
    total = np.zeros((D, TOK), dtype=np.float32)
    for c in range(NCORES):
        total += results[c]["out"].astype(np.float32)
    out = total.T + bproj.astype(np.float32)[None, :]
    return np.ascontiguousarray(out.reshape(B, Q, D), dtype=np.float32)


def kernel(x, k_cache, v_cache, Wqkv, Wproj, bproj):
    if "nc" not in _STATE:
        _STATE["nc"] = build_nc()
    nc = _STATE["nc"]
    in_maps = prepare_in_maps(
        np.asarray(x), np.asarray(k_cache), np.asarray(v_cache),
        np.asarray(Wqkv), np.asarray(Wproj)
    )
    res = run_bass_kernel_spmd(nc, in_maps, list(range(NCORES)))
    return postprocess(res.results, np.asarray(bproj))
